# revision 1
# baseline (speedup 1.0000x reference)
"""Trainium2 Bass kernel for nn_CachedAttention (8-core SPMD, tensor-parallel heads).

Contract: kernel(**inputs) takes the FULL unsharded inputs from
reference.setup_inputs() and returns the FULL (1, 2048, 2048) f32 output.

Math notes (validated against the reference in f32 at ~7e-6 rel err):
- The reference applies a TOP-LEFT-aligned causal mask tril(T, S) over the
  concatenated [cache; new] sequence, so new token t only attends to
  positions 0..t — all inside the 2048-entry cache. The freshly projected
  k/v (wk, wv, k-norm, k-rope) are therefore completely masked out and
  never computed here.
- RMSNorm's per-token scale commutes with RoPE (both linear), and q_norm_w
  folds into the RoPE cos/sin tables:
      out = q * C + swap_halves(q) * S'
      C[t,d]    = w[d] * cos(ang[t, d%64])
      S'[t,d<64]= -w[d+64] * sin(ang[t,d]);  S'[t,d>=64] = w[d-64] * sin(ang[t,d-64])
- Scores ~ N(0,1), so softmax runs without the max-subtraction pass; the
  row sum comes free from a ones-column appended to V.
- Sharding: attention is head-sharded (core c owns q heads {2c, 2c+1}, kv
  head c — perfectly balanced over the causal structure). The final wo
  projection is token-sharded: one AllToAll per head (a single direct
  exchange, no ring) reshards attention output from (all tokens, my heads)
  to (my 256 tokens, all heads); each core then computes its 256 output
  rows against the full wo and the host concatenates token blocks.

Perf notes:
- A tiny AllToAll at kernel start absorbs the large one-time collective
  arming cost, overlapped with the q-projection.
- Head-0's AllToAll overlaps head-1's attention; only head-1's exchange
  (~0.5 MB, one step) is exposed.
- ScalarE runs only Square/Sqrt/Exp (2-3 activation-table loads total);
  exp skips the fully-masked below-diagonal region.
"""

import math
import sys

import numpy as np

sys.path.insert(0, "/opt/trn_rl_repo")

import ml_dtypes

P = 128
T = 2048
DM = 2048
DK = 128
HLOC = 2          # q heads per core
NCORES = 8
NT = T // P       # 16 token tiles
ND = DM // P      # 16 contraction chunks
NS = T // P       # 16 cache s-tiles
GW = 4            # token tiles per attention group (512 wide)
NG = NT // GW     # 4 groups
NTL = T // NCORES // P   # 2 local token tiles after resharding
EPS = 1e-6
ROPE_BASE = 10000.0

_bf16 = ml_dtypes.bfloat16


def _build_module():
    import concourse.tile as tile
    from concourse import bacc, mybir

    bf = mybir.dt.bfloat16
    f32 = mybir.dt.float32
    AF = mybir.ActivationFunctionType

    nc = bacc.Bacc("TRN2", target_bir_lowering=False, debug=False, num_devices=NCORES)

    xT = nc.dram_tensor("xT", [P, 8, ND, 256], bf, kind="ExternalInput").ap()
    wqT = nc.dram_tensor("wqT", [P, ND, HLOC * DK], bf, kind="ExternalInput").ap()
    kcT = nc.dram_tensor("kcT", [DK, T], bf, kind="ExternalInput").ap()
    vca = nc.dram_tensor("vca", [P, NT, DK + 1], bf, kind="ExternalInput").ap()
    woT = nc.dram_tensor("woT", [P, HLOC, 4, NCORES, 512], bf, kind="ExternalInput").ap()
    cosw = nc.dram_tensor("cosw", [P, NT, HLOC * DK], bf, kind="ExternalInput").ap()
    sinw = nc.dram_tensor("sinw", [P, NT, HLOC * DK], bf, kind="ExternalInput").ap()
    tri = nc.dram_tensor("tri", [P, P], bf, kind="ExternalInput").ap()
    ident = nc.dram_tensor("ident", [P, P], bf, kind="ExternalInput").ap()
    out = nc.dram_tensor("out", [T // NCORES, DM], bf, kind="ExternalOutput").ap()

    with tile.TileContext(nc) as tc:
        with (
            tc.tile_pool(name="res", bufs=1) as res,
            tc.tile_pool(name="xpool", bufs=4) as xpool,
            tc.tile_pool(name="wopool", bufs=4) as wopool,
            tc.tile_pool(name="work", bufs=4) as work,
            tc.tile_pool(name="probs", bufs=20) as probs_pool,
            tc.tile_pool(name="small", bufs=6) as small,
            tc.tile_pool(name="outp", bufs=3) as outp,
            tc.tile_pool(name="ps_big", bufs=4, space="PSUM") as ps_big,
            tc.tile_pool(name="ps_tr", bufs=2, space="PSUM") as ps_tr,
            tc.tile_pool(name="ps_o", bufs=2, space="PSUM") as ps_o,
            tc.tile_pool(name="dram", bufs=1, space="DRAM") as dram,
        ):
            # ---- phase-B-critical loads first (sync-queue order ~ priority) ----
            wq_sb = res.tile([P, ND, HLOC * DK], bf)
            nc.sync.dma_start(wq_sb, wqT)
            eps_sb = res.tile([P, 1], f32)
            nc.vector.memset(eps_sb, EPS)

            # Warm up the collective path: the first collective in a NEFF
            # pays a large one-time arming cost; absorb it here, overlapped
            # with the q-projection phase.
            warm_in = dram.tile([NCORES, 16], bf, name="warm_in")
            warm_out = dram.tile([NCORES, 16], bf, name="warm_out")
            warm_sb = res.tile([NCORES, 16], bf)
            nc.vector.memset(warm_sb, 0.0)
            nc.sync.dma_start(warm_in, warm_sb)
            nc.gpsimd.collective_compute(
                "AllToAll",
                mybir.AluOpType.bypass,
                ins=[warm_in.opt()],
                outs=[warm_out.opt()],
                replica_groups=[list(range(NCORES))],
            )

            qT = [res.tile([P, T], bf, name=f"qT{h}") for h in range(HLOC)]
            att_sb = [res.tile([P, NT, DK], bf, name=f"att{h}")
                      for h in range(HLOC)]
            qr_all = res.tile([P, NT, HLOC * DK], bf)
            ssq_all = res.tile([P, NT * HLOC], f32)
            rstd_all = res.tile([P, NT * HLOC], f32)

            # ---- phase B: q projection + rope (rstd deferred) ----
            TCH = 256
            cos_sb = sin_sb = id_sb = None
            for tci in range(T // TCH):
                x_sb = xpool.tile([P, ND, TCH], bf)
                nc.sync.dma_start(x_sb, xT[:, tci])
                if tci == 0:
                    # tables are consumed later than x; load after the first
                    # x chunk so the projection matmuls start sooner
                    cos_sb = res.tile([P, NT, HLOC * DK], bf)
                    nc.sync.dma_start(cos_sb, cosw)
                    sin_sb = res.tile([P, NT, HLOC * DK], bf)
                    nc.sync.dma_start(sin_sb, sinw)
                    id_sb = res.tile([P, P], bf)
                    nc.sync.dma_start(id_sb, ident)
                for tj in range(TCH // P):
                    ti = tci * (TCH // P) + tj
                    pq = ps_big.tile([P, HLOC * DK], f32, tag="ps")
                    for dc in range(ND):
                        nc.tensor.matmul(
                            pq,
                            lhsT=x_sb[:, dc, tj * P:(tj + 1) * P],
                            rhs=wq_sb[:, dc, :],
                            start=(dc == 0),
                            stop=(dc == ND - 1),
                        )
                    qsb = work.tile([P, HLOC * DK], bf, tag="qsb")
                    nc.vector.tensor_copy(qsb, pq)
                    for h in range(HLOC):
                        idx = ti * HLOC + h
                        # sumsq on ScalarE (idle in this phase); scratch unused
                        qsq = work.tile([P, DK], bf, tag="qsq")
                        nc.scalar.activation(
                            out=qsq, in_=pq[:, h * DK:(h + 1) * DK],
                            func=AF.Square,
                            accum_out=ssq_all[:, idx:idx + 1])
                    # rope both heads at once: qr = q*C2 + swap_halves(q)*S2
                    q4 = qsb.rearrange("p (h a d) -> p h a d", h=HLOC, a=2)
                    s4 = sin_sb[:, ti, :].rearrange("p (h a d) -> p h a d",
                                                    h=HLOC, a=2)
                    u = work.tile([P, HLOC * DK], bf, tag="u")
                    u4 = u.rearrange("p (h a d) -> p h a d", h=HLOC, a=2)
                    nc.vector.tensor_mul(
                        u4[:, :, 0, :], q4[:, :, 1, :], s4[:, :, 0, :])
                    nc.vector.tensor_mul(
                        u4[:, :, 1, :], q4[:, :, 0, :], s4[:, :, 1, :])
                    t1 = work.tile([P, HLOC * DK], bf, tag="t1")
                    nc.vector.tensor_mul(t1, qsb, cos_sb[:, ti, :])
                    nc.vector.tensor_add(qr_all[:, ti, :], t1, u)

            # batched rstd: one Sqrt + one reciprocal for all 32 (ti, h)
            nc.scalar.activation(
                out=ssq_all, in_=ssq_all, func=AF.Sqrt,
                bias=eps_sb, scale=1.0 / DK)
            nc.vector.reciprocal(rstd_all, ssq_all)

            for h in range(HLOC):
                for ti in range(NT):
                    idx = ti * HLOC + h
                    qrs = work.tile([P, DK], bf, tag="qrs")
                    nc.vector.tensor_scalar_mul(
                        qrs, qr_all[:, ti, h * DK:(h + 1) * DK],
                        rstd_all[:, idx:idx + 1])
                    ptr = ps_tr.tile([P, P], bf, tag="ptr")
                    nc.tensor.transpose(ptr, qrs, id_sb)
                    nc.vector.tensor_copy(qT[h][:, ti * P:(ti + 1) * P], ptr)

            # ---- attention-phase loads ----
            kc_sb = res.tile([P, T], bf)
            nc.sync.dma_start(kc_sb, kcT)
            vca_sb = res.tile([P, NS, DK + 1], bf)
            nc.sync.dma_start(vca_sb, vca)
            tri_sb = res.tile([P, P], bf)
            nc.sync.dma_start(tri_sb, tri)

            # ---- phase C: attention; each head's AllToAll right after it ----
            # attT[h][:, r, w, :] = head-h attention output, TRANSPOSED to
            # [dk, tok] for token tile 2r+w — transposing on the SENDER (PE
            # has slack in the exp-paced attention phase) means the a2a
            # payload lands in wo-lhsT layout and the receive path is a
            # single plain DMA with NO post-collective transposes.
            attT = [res.tile([P, NCORES, NTL, P], bf, name=f"attT{h}")
                    for h in range(HLOC)]
            aoT = [res.tile([P, NCORES, NTL * P], bf, name=f"aoT{h}")
                   for h in range(HLOC)]
            a_in = [dram.tile([NCORES * DK, NTL * P], bf, name=f"a_in{h}")
                    for h in range(HLOC)]
            a_out = [dram.tile([NCORES * DK, NTL * P], bf, name=f"a_out{h}")
                     for h in range(HLOC)]
            a_in_r = [a_in[h].rearrange("(r p) t -> p r t", p=P)
                      for h in range(HLOC)]

            for h in range(HLOC):
                for g in range(NG):
                    t0 = g * GW * P
                    pb_tiles = []
                    for si in range(GW * (g + 1)):
                        k = max(0, si - g * GW)  # skip below-diagonal tiles
                        ps = ps_big.tile([P, GW * P], f32, tag="ps")
                        nc.tensor.matmul(
                            ps[:, k * P:],
                            lhsT=kc_sb[:, si * P:(si + 1) * P],
                            rhs=qT[h][:, t0 + k * P:t0 + GW * P],
                            start=True, stop=True,
                        )
                        pb = probs_pool.tile([P, GW * P], bf, tag="pb")
                        nc.scalar.activation(
                            out=pb[:, k * P:], in_=ps[:, k * P:], func=AF.Exp)
                        if si >= g * GW:
                            nc.vector.tensor_mul(
                                pb[:, k * P:(k + 1) * P],
                                pb[:, k * P:(k + 1) * P], tri_sb)
                        pb_tiles.append(pb)
                    for tj in range(GW):
                        ti = g * GW + tj
                        po = ps_o.tile([P, DK + 1], f32, tag="po")
                        for si in range(ti + 1):
                            nc.tensor.matmul(
                                po,
                                lhsT=pb_tiles[si][:, tj * P:(tj + 1) * P],
                                rhs=vca_sb[:, si, :],
                                start=(si == 0), stop=(si == ti),
                            )
                        recip = small.tile([P, 1], f32, tag="recip")
                        nc.vector.reciprocal(recip, po[:, DK:DK + 1])
                        nc.vector.tensor_scalar_mul(
                            att_sb[h][:, ti, :], po[:, :DK], recip)
                        ptr3 = ps_tr.tile([P, P], bf, tag="ptr")
                        nc.tensor.transpose(ptr3, att_sb[h][:, ti, :], id_sb)
                        nc.vector.tensor_copy(
                            attT[h][:, ti // NTL, ti % NTL, :], ptr3)
                    # stage this group's chunks of the exchange payload
                    nc.sync.dma_start(
                        a_in_r[h][:, g * NTL:(g + 1) * NTL, :],
                        attT[h][:, g * NTL:(g + 1) * NTL].rearrange(
                            "p r w t -> p r (w t)"))

                # AllToAll head h: chunk r = [dk, 256 tok] for rank r; the
                # received block i is already the wo-chain lhsT for head 2i+h
                nc.gpsimd.collective_compute(
                    "AllToAll",
                    mybir.AluOpType.bypass,
                    ins=[a_in[h].opt()],
                    outs=[a_out[h].opt()],
                    replica_groups=[list(range(NCORES))],
                )
                nc.sync.dma_start(
                    aoT[h], a_out[h].rearrange("(i p) t -> p i t", p=P))

            # ---- phase E: wo chains with deferred head-1 halves ----
            WCH = 512
            NCH = DM // WCH
            out_r = out.rearrange("(tj p) f -> p tj f", p=P)

            def wo_load(h, nch):
                wos = wopool.tile([P, NCORES, WCH], bf, tag="wo",
                                  name=f"wo{h}_{nch}")
                nc.sync.dma_start(wos, woT[:, h, nch])
                return wos

            def half_chain(pout, h, wos, tj, start, stop):
                for i in range(NCORES):
                    nc.tensor.matmul(
                        pout,
                        lhsT=aoT[h][:, i, tj * P:(tj + 1) * P],
                        rhs=wos[:, i, :],
                        start=(start and i == 0),
                        stop=(stop and i == NCORES - 1),
                    )

            def finish(pout, nch, tj):
                osb = outp.tile([P, WCH], f32, tag="osb")
                nc.vector.tensor_copy(osb, pout)
                nc.sync.dma_start(
                    out_r[:, tj, nch * WCH:(nch + 1) * WCH], osb)

            # ALL head-0 half-chains run as soon as aoT[0] lands (during the
            # head-1 AllToAll wait), each completed (stop=True) and stashed
            # to SBUF in f32 so the PSUM ring stays free.  After aoT[1],
            # only the head-1 halves remain; the stash folds in during the
            # output evacuation (tensor_add replaces tensor_copy — no extra
            # op on the tail).
            stash = {}
            for nch in range(NCH):
                wos0 = wo_load(0, nch)
                for tj in range(NTL):
                    pout = ps_big.tile([P, WCH], f32, tag="ps")
                    half_chain(pout, 0, wos0, tj, True, True)
                    st = outp.tile([P, WCH], f32, tag="st", name=f"st{nch}_{tj}",
                                   bufs=NCH * NTL)
                    nc.vector.tensor_copy(st, pout)
                    stash[(nch, tj)] = st

            for nch in range(NCH):
                wos1 = wo_load(1, nch)
                for tj in range(NTL):
                    pout = ps_big.tile([P, WCH], f32, tag="ps")
                    half_chain(pout, 1, wos1, tj, True, True)
                    osb = outp.tile([P, WCH], bf, tag="osb")
                    nc.vector.tensor_add(osb, pout, stash[(nch, tj)])
                    nc.sync.dma_start(
                        out_r[:, tj, nch * WCH:(nch + 1) * WCH], osb)

    nc.compile()
    return nc


def _host_inputs(x, cached_k, cached_v, wq, wo, q_norm_w):
    """Build the 8 per-core input maps (host-side shard + fold + cast).

    All device tensors are pre-arranged so every DMA line is a long
    contiguous run per partition (>=4KB)."""
    xt = np.ascontiguousarray(x[0].T).astype(np.float32)          # (DM, T)
    # x_prep[p, c, o, t'] = xT[o*128+p, c*256+t']
    x_prep = np.ascontiguousarray(
        xt.reshape(ND, P, 8, 256).transpose(1, 2, 0, 3)).astype(_bf16)

    wot = np.ascontiguousarray(wo.T).astype(np.float32)           # (DM, DM)
    # wo_prep[p, h, c, i, f] = woT[(2i+h)*128 + p, c*512 + f]
    wo_prep = np.ascontiguousarray(
        wot.reshape(NCORES, HLOC, P, 4, 512).transpose(2, 1, 3, 0, 4)
    ).astype(_bf16)

    inv_freq = 1.0 / (ROPE_BASE ** (np.arange(0, DK, 2, dtype=np.float32) / DK))
    ang = np.arange(T, dtype=np.float32)[:, None] * inv_freq[None, :]
    cos_f = np.concatenate([np.cos(ang), np.cos(ang)], axis=1)
    sin_f = np.concatenate([np.sin(ang), np.sin(ang)], axis=1)
    w = q_norm_w.astype(np.float32)
    C = (w[None, :] * cos_f).astype(np.float32)
    Sp = np.empty((T, DK), np.float32)
    Sp[:, :DK // 2] = -w[None, DK // 2:] * sin_f[:, :DK // 2]
    Sp[:, DK // 2:] = w[None, :DK // 2] * sin_f[:, DK // 2:]
    C2 = np.tile(C, (1, HLOC))    # (T, 256) both heads
    S2 = np.tile(Sp, (1, HLOC))
    # [p, ti, d] = tab[ti*128 + p, d]
    C2p = np.ascontiguousarray(
        C2.reshape(NT, P, HLOC * DK).transpose(1, 0, 2)).astype(_bf16)
    S2p = np.ascontiguousarray(
        S2.reshape(NT, P, HLOC * DK).transpose(1, 0, 2)).astype(_bf16)

    tri_m = (np.arange(P)[:, None] <= np.arange(P)[None, :]).astype(_bf16)
    ident = np.eye(P, dtype=_bf16)

    in_maps = []
    for c in range(NCORES):
        fs = slice(c * HLOC * DK, (c + 1) * HLOC * DK)
        wqt = np.ascontiguousarray(wq[fs, :].T).astype(np.float32)  # (DM, 256)
        wq_prep = np.ascontiguousarray(
            wqt.reshape(ND, P, HLOC * DK).transpose(1, 0, 2)).astype(_bf16)
        kcT = np.ascontiguousarray(cached_k[c].T / math.sqrt(DK)).astype(_bf16)
        vcaa = np.concatenate(
            [cached_v[c], np.ones((T, 1), np.float32)], axis=1)
        vca_prep = np.ascontiguousarray(
            vcaa.reshape(NT, P, DK + 1).transpose(1, 0, 2)).astype(_bf16)
        in_maps.append({
            "xT": x_prep, "wqT": wq_prep, "kcT": kcT, "vca": vca_prep,
            "woT": wo_prep, "cosw": C2p, "sinw": S2p, "tri": tri_m,
            "ident": ident,
        })
    return in_maps


_CACHED = {}


def _get_module():
    if "nc" not in _CACHED:
        _CACHED["nc"] = _build_module()
    return _CACHED["nc"]


def run(inputs, trace=False, **kw):
    """Compile (cached), run on 8 cores, return (output, BassKernelResults)."""
    from concourse import bass_utils

    nc = _get_module()
    in_maps = _host_inputs(
        np.asarray(inputs["x"], np.float32),
        np.asarray(inputs["cached_k"], np.float32),
        np.asarray(inputs["cached_v"], np.float32),
        np.asarray(inputs["wq"], np.float32),
        np.asarray(inputs["wo"], np.float32),
        np.asarray(inputs["q_norm_w"], np.float32),
    )
    res = bass_utils.run_bass_kernel_spmd(
        nc, in_maps, core_ids=list(range(NCORES)), trace=trace, **kw)
    rows = [res.results[c]["out"] for c in range(NCORES)]
    full = np.concatenate(rows, axis=0).reshape(1, T, DM).astype(np.float32)
    return full, res


def kernel(**inputs):
    full, _ = run(inputs)
    return full

